# revision 41
# baseline (speedup 1.0000x reference)
"""CrossAttention (cosine-sim, learnable temperature) Trainium2 kernel, v2.

Math (per batch element b, reference in fp32):
    qh  = (q @ Wq.T)   -> [Lq, C] -> heads [H, Lq, D]
    k,v = (kv @ Wkv.T) -> k,v [H, Lkv, D]
    qn = qh / ||qh||_d; kn = k / ||k||_d
    attn = softmax(qn @ kn.T / tau); out = attn @ v
    y = out @ Wproj.T + bproj         (bproj added on host)

Distribution: pure data-parallel over B=8 across the 8 NeuronCores (one
batch element per core, weights replicated, no collectives).

v2 design notes (changes vs v1 baseline, driven by the NTFF trace):
  * DVE `reciprocal` was 3.3us/instr (213us total, serializing both
    phases).  Replaced with `reciprocal_approx_fast` (~0.66us, fp32).
  * eps-add and tau fold into the ACT Sqrt (bias / tau^2 pre-scale), so
    the norm chain is Square -> ones-matmul -> Sqrt -> fast-recip.
  * The k-side normalization (rk/tau) is applied inside the softmax Exp
    as a per-partition (lkv) activation scale instead of scaling knT.
    Needs rk transposed to [lkv, h]: 64 tiny PE transposes ([2,128] ->
    [128,2]) during phase 1a.  Saves the k-side broadcast matmuls,
    evacuations and multiplies entirely.
  * Softmax-sum reciprocal also via fast-recip (fp32); the broadcast
    matmuls run in f32r (full rate at free-size >= 256).
  * Output projection bias is added on the host; bias matmuls dropped.
  * Phase 2 emission interleaves scores(pair i+1) with PV(pair i) at
    kt granularity so the in-order PE queue never drains while ACT
    works through the Exp stream (the PE HAM clock-gate only reaches
    2.4 GHz when the engine stays busy; idle windows re-throttle it
    to 1.2 GHz).
  * V-projection fully in phase 1a (interleaved with K jobs); O-proj
    first half (ct 0-3) interleaved into pairs 4-7 via y_mid, second
    half in the tail.
"""

import sys

sys.path.insert(0, "/opt/trn_rl_repo")

import numpy as np
import ml_dtypes

import concourse.bass as bass
import concourse.bacc as bacc
import concourse.mybir as mybir
from concourse.tile import TileContext
from concourse.bass_utils import run_bass_kernel_spmd

AF = mybir.ActivationFunctionType
F32 = mybir.dt.float32
F32R = mybir.dt.float32r
F16 = mybir.dt.float16
BF16 = mybir.dt.bfloat16

NCORES = 8


def r32(ap):
    """fp32 AP -> float32r view (full-rate PE matmul on fp32 data)."""
    return ap.bitcast(F32R)


DEFAULT_KNOBS = dict(
    psA_bufs=4, psS_bufs=2, psB_bufs=2,
    sq_bufs=3, smalls_bufs=4, rbs_bufs=2,
    psSc_bufs=2, psPV_bufs=3, psBc_bufs=1,
    pt_bufs=4, rsum_bufs=2, sbb_bufs=3, tmp_bufs=2, y_bufs=2,
)


def build_nc(C=1024, H=16, LQ=1024, LKV=1024, knobs=None):
    kb = dict(DEFAULT_KNOBS)
    if knobs:
        kb.update(knobs)
    P = 128
    D = C // H            # head dim (64)
    OT = C // P           # feature tiles (8)
    CT = C // P           # contraction tiles (8)
    KT = LKV // P         # lkv partition tiles (8)
    HPT = P // D          # heads per 128-tile (2)
    CH = min(512, LQ)     # free-dim chunk per psum bank (fp32)
    NCH = LQ // CH        # chunks of Lq (2)
    VCH = min(512, C)     # chunk of output features for V projection
    NVCH = C // VCH
    HPC = VCH // D        # heads per v-projection chunk (8)

    nc = bacc.Bacc("TRN2", target_bir_lowering=False)

    qT = nc.dram_tensor("qT", [C, LQ], F16, kind="ExternalInput")
    kvT = nc.dram_tensor("kvT", [C, LKV], F16, kind="ExternalInput")
    wqT = nc.dram_tensor("wqT", [C, C], F16, kind="ExternalInput")
    wkT = nc.dram_tensor("wkT", [C, C], F16, kind="ExternalInput")
    wvT = nc.dram_tensor("wvT", [C, C], F16, kind="ExternalInput")
    wpT = nc.dram_tensor("wpT", [C, C], BF16, kind="ExternalInput")
    tau2 = nc.dram_tensor("tau2", [HPT, 1], F32, kind="ExternalInput")
    ones_blk = nc.dram_tensor("ones_blk", [P, HPT], F16, kind="ExternalInput")
    blk2 = nc.dram_tensor("blk2", [HPT, P], F16, kind="ExternalInput")
    y = nc.dram_tensor("y", [LQ, C], F32, kind="ExternalOutput")

    qT_r = qT.rearrange("(ct p) l -> p ct l", p=P)
    kvT_r = kvT.rearrange("(ct p) l -> p ct l", p=P)
    wqT_r = wqT.rearrange("(ct p) o -> p ct o", p=P)
    wkT_r = wkT.rearrange("(ct p) o -> p ct o", p=P)
    wvT_r = wvT.rearrange("(ct p) o -> p ct o", p=P)
    wpT_r = wpT.rearrange("(ct p) o -> p ct o", p=P)
    y_r = y.rearrange("(yt p) o -> p yt o", p=P)

    with TileContext(nc) as tc:
        from contextlib import ExitStack

        with ExitStack() as stk:
            # ---------- persistent pools --------------------------------
            persist = stk.enter_context(tc.tile_pool(name="persist", bufs=1))
            qnT = persist.tile([P, OT, LQ], F16)            # qh * rq
            knT = persist.tile([P, OT, LKV], F16)           # raw kh (unnormalized)
            v_aug = persist.tile([P, KT, H, D + 1], BF16)   # [v | ones]
            oT = persist.tile([P, CT, LQ], BF16)            # (attn@v)/sum
            wp_sb = persist.tile([P, CT, C], BF16)
            consts = stk.enter_context(tc.tile_pool(name="consts", bufs=1))
            ones_blk_sb = consts.tile([P, HPT], F16)
            blk2_sb = consts.tile([HPT, P], F16)
            tau2_sb = consts.tile([HPT, 1], F32)
            ones64 = consts.tile([1, D], BF16)

            nc.sync.dma_start(out=ones_blk_sb, in_=ones_blk[:, :])
            nc.sync.dma_start(out=blk2_sb, in_=blk2[:, :])
            nc.sync.dma_start(out=tau2_sb, in_=tau2[:, :])
            nc.vector.memset(ones64, 1.0)
            nc.vector.memset(v_aug[:, :, :, D : D + 1], 1.0)

            # ---------- phase 1 (scoped so pools free before phase 2) ----
            p1 = ExitStack()
            # Interleave kv-chunk / wk-column / wv-column DMAs so the first
            # K job unblocks as soon as ~3 chunks have landed, then q + q
            # weights (phase 1b), O-proj weights last.
            p1w = p1.enter_context(tc.tile_pool(name="p1w", bufs=1))
            kvT_sb = p1w.tile([P, CT, LKV], F16)
            wk_sb = p1w.tile([P, CT, C], F16)
            wv_sb = p1w.tile([P, CT, VCH], F16)   # first half only (vch 0)
            qT_sb = p1w.tile([P, CT, LQ], F16)
            wq_sb = p1w.tile([P, CT, C], F16)
            # ch0 halves of kv first (the ch0 K jobs need only those), then
            # the wv columns, then the ch1 halves.
            for ct in range(CT):
                sl = slice(ct * P, (ct + 1) * P)
                nc.sync.dma_start(out=kvT_sb[:, ct, 0:CH], in_=kvT_r[:, ct, 0:CH])
                nc.sync.dma_start(out=wk_sb[:, :, sl], in_=wkT_r[:, :, sl])
            for ct in range(VCH // P):
                sl = slice(ct * P, (ct + 1) * P)
                nc.sync.dma_start(out=wv_sb[:, :, sl], in_=wvT_r[:, :, sl])
            for ct in range(CT):
                nc.sync.dma_start(
                    out=kvT_sb[:, ct, CH:LKV], in_=kvT_r[:, ct, CH:LKV]
                )
            for ct in range(CT):
                sl = slice(ct * P, (ct + 1) * P)
                nc.sync.dma_start(out=qT_sb[:, ct, :], in_=qT_r[:, ct, :])
                nc.sync.dma_start(out=wq_sb[:, :, sl], in_=wqT_r[:, :, sl])
            for ct in range(CT):
                nc.sync.dma_start(out=wp_sb[:, ct, :], in_=wpT_r[:, ct, :])

            # ============ PHASE 1a: K norm-proj + V proj ================
            class Job:
                def A(self):
                    pass

                def B(self):
                    pass

                def Cs(self):
                    pass

            def run_pipeline(jobs):
                n = len(jobs)
                for i in range(n + 2):
                    if i < n:
                        jobs[i].A()
                    if 0 <= i - 1 < n:
                        jobs[i - 1].B()
                    if 0 <= i - 2 < n:
                        jobs[i - 2].Cs()

            with ExitStack() as p1c:
                sqp = p1c.enter_context(tc.tile_pool(name="sqp", bufs=kb["sq_bufs"]))
                smalls = p1c.enter_context(
                    tc.tile_pool(name="smalls", bufs=kb["smalls_bufs"])
                )
                rbs = p1c.enter_context(tc.tile_pool(name="rbsa", bufs=kb["rbs_bufs"]))
                psA = p1c.enter_context(
                    tc.tile_pool(name="psA", bufs=kb["psA_bufs"], space="PSUM")
                )
                psS = p1c.enter_context(
                    tc.tile_pool(name="psS", bufs=kb["psS_bufs"], space="PSUM")
                )
                psB = p1c.enter_context(
                    tc.tile_pool(name="psB", bufs=kb["psB_bufs"], space="PSUM")
                )

                class KJob(Job):
                    def __init__(self, ot, ch):
                        self.ot, self.ch = ot, ch
                        self.sl = slice(ch * CH, (ch + 1) * CH)

                    def A(self):
                        self.ph = psA.tile([P, CH], F32, tag="ph", name="ph")
                        wcol = wk_sb[:, :, self.ot * P : (self.ot + 1) * P]
                        for ct in range(CT):
                            nc.tensor.matmul(
                                self.ph,
                                wcol[:, ct, :],
                                kvT_sb[:, ct, self.sl],
                                start=(ct == 0),
                                stop=(ct == CT - 1),
                            )
                        self.sq = sqp.tile([P, CH], F16, tag="sq", name="sq")
                        nc.scalar.activation(self.sq, self.ph, AF.Square)

                    def B(self):
                        ssq = psS.tile([HPT, CH], F32, tag="ssq", name="ssq")
                        nc.tensor.matmul(ssq, ones_blk_sb, self.sq, start=True, stop=True)
                        # rr = sqrt(ssq * tau^2) = tau * ||kh||  (f16, so the
                        # broadcast matmul below runs at full f16 rate)
                        self.rr = smalls.tile([HPT, CH], F16, tag="rr", name="rr")
                        nc.scalar.activation(self.rr, ssq, AF.Sqrt, scale=tau2_sb)

                    def Cs(self):
                        rb = psB.tile([P, CH], F32, tag="rb", name="rb")
                        nc.tensor.matmul(rb, blk2_sb, self.rr, start=True, stop=True)
                        rb_sb = rbs.tile([P, CH], F32, tag="rb_sb", name="rb_sb")
                        nc.vector.reciprocal_approx_fast(rb_sb, rb)
                        nc.vector.tensor_mul(
                            knT[:, self.ot, self.sl], self.ph, rb_sb
                        )

                class VJob(Job):
                    def __init__(self, vch, vt):
                        self.vch, self.vt = vch, vt

                    def A(self):
                        self.pv = psA.tile([P, VCH], F32, tag="ph", name="pv")
                        wcol = wv_sb[:, :, self.vch * VCH : (self.vch + 1) * VCH]
                        for ct in range(CT):
                            nc.tensor.matmul(
                                self.pv,
                                kvT_sb[:, ct, self.vt * P : (self.vt + 1) * P],
                                wcol[:, ct, :],
                                start=(ct == 0),
                                stop=(ct == CT - 1),
                            )

                    def Cs(self):
                        nc.vector.tensor_copy(
                            v_aug[
                                :, self.vt, self.vch * HPC : (self.vch + 1) * HPC, 0:D
                            ],
                            self.pv.rearrange("p (h d) -> p h d", d=D),
                        )

                class QJob(Job):
                    def __init__(self, ot, ch):
                        self.ot, self.ch = ot, ch
                        self.sl = slice(ch * CH, (ch + 1) * CH)

                    def A(self):
                        self.ph = psA.tile([P, CH], F32, tag="ph", name="ph")
                        wcol = wq_sb[:, :, self.ot * P : (self.ot + 1) * P]
                        for ct in range(CT):
                            nc.tensor.matmul(
                                self.ph,
                                wcol[:, ct, :],
                                qT_sb[:, ct, self.sl],
                                start=(ct == 0),
                                stop=(ct == CT - 1),
                            )
                        self.sq = sqp.tile([P, CH], F16, tag="sq", name="sq")
                        nc.scalar.activation(self.sq, self.ph, AF.Square)

                    def B(self):
                        ssq = psS.tile([HPT, CH], F32, tag="ssq", name="ssq")
                        nc.tensor.matmul(ssq, ones_blk_sb, self.sq, start=True, stop=True)
                        # ||qh|| in f16, broadcast BEFORE the reciprocal so the
                        # broadcast matmul runs in f16 (no f32r rounding issue)
                        self.rr = smalls.tile([HPT, CH], F16, tag="rr", name="rr")
                        nc.scalar.activation(self.rr, ssq, AF.Sqrt)

                    def Cs(self):
                        rb = psB.tile([P, CH], F32, tag="rb", name="rb")
                        nc.tensor.matmul(rb, blk2_sb, self.rr, start=True, stop=True)
                        rb_sb = rbs.tile([P, CH], F32, tag="rb_sb", name="rb_sb")
                        nc.vector.reciprocal_approx_fast(rb_sb, rb)
                        nc.vector.tensor_mul(
                            qnT[:, self.ot, self.sl], self.ph, rb_sb
                        )

                # One merged pipeline: ch0 K jobs first (they only need the
                # ch0 kv halves), V jobs slotted in as their inputs land,
                # then ch1 K jobs, then all Q jobs.  A single pool scope
                # means no pipeline drain at the K/Q boundary.
                jobs = [KJob(ot, 0) for ot in range(4)]
                for i in range(4):
                    jobs += [KJob(4 + i, 0), VJob(0, i)]
                for i in range(4):
                    jobs += [KJob(i, 1), VJob(0, 4 + i)]
                jobs += [KJob(4 + i, 1) for i in range(4)]
                jobs += [QJob(i // 2, i % 2) for i in range(2 * OT)]
                run_pipeline(jobs)

            # free phase-1 inputs/weights before the big pt pool allocates
            p1.close()

            # ============ PHASE 2: attention (head pairs) ===============
            with ExitStack() as p2:
                ymp = p2.enter_context(tc.tile_pool(name="ymp", bufs=1))
                y_mid = ymp.tile([P, LQ // P, C], BF16)
                wv1p = p2.enter_context(tc.tile_pool(name="wv1p", bufs=1))
                wv1_sb = wv1p.tile([P, CT, VCH], F16)
                for ct in range(CT):
                    nc.sync.dma_start(
                        out=wv1_sb[:, ct, :], in_=wvT_r[:, ct, VCH : 2 * VCH]
                    )
                # prefetch all kv blocks for the V-proj second half now
                kvbp = p2.enter_context(tc.tile_pool(name="kvbp", bufs=KT))
                kvb_tiles = []
                for vt in range(KT):
                    kvb = kvbp.tile([P, CT, P], F16, tag="kvb", name="kvb")
                    nc.sync.dma_start(
                        out=kvb, in_=kvT_r[:, :, vt * P : (vt + 1) * P]
                    )
                    kvb_tiles.append(kvb)
                ptp = p2.enter_context(tc.tile_pool(name="ptp", bufs=kb["pt_bufs"]))
                rsp = p2.enter_context(tc.tile_pool(name="rsp", bufs=kb["rsum_bufs"]))
                sbb = p2.enter_context(tc.tile_pool(name="sbb", bufs=kb["sbb_bufs"]))
                tmpp = p2.enter_context(tc.tile_pool(name="tmpp", bufs=kb["tmp_bufs"]))
                yp = p2.enter_context(tc.tile_pool(name="yp", bufs=kb["y_bufs"]))
                psSc = p2.enter_context(
                    tc.tile_pool(name="psSc", bufs=kb["psSc_bufs"], space="PSUM")
                )
                psPV = p2.enter_context(
                    tc.tile_pool(name="psPV", bufs=kb["psPV_bufs"], space="PSUM")
                )
                psBc = p2.enter_context(
                    tc.tile_pool(name="psBc", bufs=kb["psBc_bufs"], space="PSUM")
                )

                def emit_scores_step(ot, kt, pt0, pt1):
                    """One kt slice of a head pair's scores + exp.  The two
                    matmuls sit on PE row groups 0-1 / 2-3 (base partitions
                    0 and 64) and execute concurrently on hardware."""
                    r0 = slice(0, D)
                    r1 = slice(D, 2 * D)
                    kl = slice(kt * P, (kt + 1) * P)
                    s0 = psSc.tile([P, LQ], F32, tag="ps_s", name="s0")
                    s1 = psSc.tile([P, LQ], F32, tag="ps_s", name="s1")
                    for ch in range(NCH):
                        sl = slice(ch * CH, (ch + 1) * CH)
                        nc.tensor.matmul(
                            s0[:, sl], knT[r0, ot, kl], qnT[r0, ot, sl],
                            start=True, stop=True,
                        )
                        nc.tensor.matmul(
                            s1[:, sl], knT[r1, ot, kl], qnT[r1, ot, sl],
                            start=True, stop=True,
                        )
                    nc.scalar.activation(pt0[:, kt, :], s0, AF.Exp)
                    nc.scalar.activation(pt1[:, kt, :], s1, AF.Exp)

                def pv_mms(h, ch, pt):
                    """attn@v (+softmax sum via the ones column) matmuls for
                    one (head, Lq-chunk)."""
                    sl = slice(ch * CH, (ch + 1) * CH)
                    pv = psPV.tile([D + 1, CH], F32, tag="ps_pv", name="ps_pv")
                    for kt in range(KT):
                        nc.tensor.matmul(
                            pv,
                            v_aug[:, kt, h, :],
                            pt[:, kt, sl],
                            start=(kt == 0),
                            stop=(kt == KT - 1),
                        )
                    return pv

                def pv_tail(h, ch, pv):
                    """Softmax-sum fast-recip, broadcast, normalize into oT."""
                    par, ot = h % HPT, h // HPT
                    sl = slice(ch * CH, (ch + 1) * CH)
                    sums = rsp.tile([1, CH], BF16, tag="rsum", name="sums")
                    nc.vector.tensor_copy(sums, pv[D : D + 1, :])
                    ps_b = psBc.tile([D, CH], F32, tag="ps_b", name="ps_b")
                    nc.tensor.matmul(ps_b, ones64, sums, start=True, stop=True)
                    sb_b = sbb.tile([D, CH], F32, tag="sb_b", name="sb_b")
                    nc.vector.reciprocal_approx_fast(sb_b, ps_b)
                    rows = slice(par * D, (par + 1) * D)
                    if par == 0:
                        nc.vector.tensor_mul(oT[rows, ot, sl], pv[0:D, :], sb_b)
                    else:
                        tmp = tmpp.tile([D, CH], BF16, tag="tmp", name="tmp")
                        nc.vector.tensor_mul(tmp, pv[0:D, :], sb_b)
                        nc.sync.dma_start(out=oT[rows, ot, sl], in_=tmp)

                def emit_vproj2(vt):
                    """Second-half V projection (heads HPC..2*HPC-1) as PE
                    filler in early pairs; kv block prefetched from DRAM."""
                    pv = psPV.tile([P, VCH], F32, tag="ps_pv", name="pv2")
                    for ct in range(CT):
                        nc.tensor.matmul(
                            pv,
                            kvb_tiles[vt][:, ct, :],
                            wv1_sb[:, ct, :],
                            start=(ct == 0),
                            stop=(ct == CT - 1),
                        )
                    nc.vector.tensor_copy(
                        v_aug[:, vt, HPC : 2 * HPC, 0:D],
                        pv.rearrange("p (h d) -> p h d", d=D),
                    )

                def emit_oproj(u, ct0, ct1, mode):
                    """Partial O-projection over ct0..ct1-1 for unit u.
                    mode: 'init' writes y_mid, 'accum' adds to it, 'final'
                    adds the last partial and DMAs the row out."""
                    yt, vch = divmod(u, NVCH)
                    sl = slice(vch * VCH, (vch + 1) * VCH)
                    ps = psPV.tile([P, VCH], F32, tag="ps_pv", name="ps_o")
                    for ct in range(ct0, ct1):
                        nc.tensor.matmul(
                            ps,
                            oT[:, ct, yt * P : (yt + 1) * P],
                            wp_sb[:, ct, sl],
                            start=(ct == ct0),
                            stop=(ct == ct1 - 1),
                        )
                    if mode == "init":
                        nc.vector.tensor_copy(y_mid[:, yt, sl], ps)
                    elif mode == "accum":
                        nc.vector.tensor_add(y_mid[:, yt, sl], ps, y_mid[:, yt, sl])
                    else:
                        y_sb = yp.tile([P, VCH], F32, tag="y_sb", name="y_sb")
                        nc.vector.tensor_add(y_sb, ps, y_mid[:, yt, sl])
                        nc.sync.dma_start(out=y_r[:, yt, sl], in_=y_sb)

                NPAIR = H // 2
                nunits = (LQ // P) * NVCH      # 16 O-proj units per ct-range

                # PE filler per pair (keeps the HAM clock-gate warm while the
                # ACT engine works through the Exp stream):
                #   pairs 0-1: V-proj second half (4 lkv tiles each)
                #   pairs 2-4: O-proj ct 0-1 init   (needs pairs 0-1 done)
                #   pairs 5-7: O-proj ct 2-3 accum  (needs pairs 2-3 done)
                #   tail:      O-proj ct 4-7 + y writeout
                filler = {pi: [] for pi in range(NPAIR)}
                for vt in range(KT):
                    filler[vt // 4].append(lambda vt=vt: emit_vproj2(vt))
                for u in range(nunits):
                    filler[2 + u // 6].append(
                        lambda u=u: emit_oproj(u, 0, 2, "init")
                    )
                    filler[5 + u // 6].append(
                        lambda u=u: emit_oproj(u, 2, 4, "accum")
                    )

                def pv_steps_for(pair, pts):
                    """PV units software-pipelined: unit j's (DVE-gated) tail
                    is emitted after unit j+1's matmuls so the in-order PE
                    queue never waits on the sum-reciprocal chain."""
                    units = [(h, ch, pt) for h, pt in zip(pair, pts)
                             for ch in range(NCH)]
                    steps, prev = [], []

                    def mk_mms(h, ch, pt, slot):
                        def f():
                            slot.append(pv_mms(h, ch, pt))
                        return f

                    def mk_tail(h, ch, slot):
                        return lambda: pv_tail(h, ch, slot[0])

                    slots = [[] for _ in units]
                    for j, (h, ch, pt) in enumerate(units):
                        steps.append(mk_mms(h, ch, pt, slots[j]))
                        if j > 0:
                            hp, cp, _ = units[j - 1]
                            steps.append(mk_tail(hp, cp, slots[j - 1]))
                    hl, cl, _ = units[-1]
                    steps.append(mk_tail(hl, cl, slots[-1]))
                    return steps

                pend = None   # steps of the previous pair's PV work
                for pi in range(NPAIR):
                    pair = (2 * pi, 2 * pi + 1)
                    ot = pi
                    pt0 = ptp.tile([P, KT, LQ], BF16, tag="pt", name="pt0")
                    pt1 = ptp.tile([P, KT, LQ], BF16, tag="pt", name="pt1")
                    psteps = (pend or []) + filler[pi]
                    np_done = 0
                    for kt in range(KT):
                        emit_scores_step(ot, kt, pt0, pt1)
                        want = (kt + 1) * len(psteps) // KT
                        while np_done < want:
                            psteps[np_done]()
                            np_done += 1
                    while np_done < len(psteps):
                        psteps[np_done]()
                        np_done += 1
                    pend = pv_steps_for(pair, (pt0, pt1))
                for s in pend:
                    s()

                # ============ PHASE 3: O-projection tail ================
                for u in range(nunits):
                    emit_oproj(u, CT // 2, CT, "final")

    nc.finalize()
    return nc


_NC_CACHE = {}


def _get_nc(C, H, LQ, LKV, knobs=None):
    key = (C, H, LQ, LKV, tuple(sorted((knobs or {}).items())))
    if key not in _NC_CACHE:
        _NC_CACHE[key] = build_nc(C, H, LQ, LKV, knobs=knobs)
    return _NC_CACHE[key]


def _host_inputs(q, kv, Wq, Wkv, Wproj, bproj, tau, H):
    B, LQ, C = q.shape
    P, D = 128, C // H
    HPT = P // D

    f16 = lambda a: np.ascontiguousarray(
        np.asarray(a, dtype=np.float32).astype(np.float16)
    )
    bf16 = lambda a: np.ascontiguousarray(
        np.asarray(a, dtype=np.float32).astype(ml_dtypes.bfloat16)
    )
    f32 = lambda a: np.ascontiguousarray(np.asarray(a, dtype=np.float32))

    wqT = f16(np.asarray(Wq).T)
    wkT = f16(np.asarray(Wkv)[:C].T)
    wvT = f16(np.asarray(Wkv)[C:].T)
    wpT = bf16(np.asarray(Wproj).T)
    tau2 = np.full((HPT, 1), float(np.asarray(tau)) ** 2, dtype=np.float32)
    ones_blk = np.zeros((P, HPT), dtype=np.float16)
    for p in range(P):
        ones_blk[p, p // D] = 1.0
    blk2 = np.ascontiguousarray(ones_blk.T)

    shared = {
        "wqT": wqT, "wkT": wkT, "wvT": wvT, "wpT": wpT,
        "tau2": tau2, "ones_blk": ones_blk, "blk2": blk2,
    }
    qn = np.asarray(q, dtype=np.float32)
    kvn = np.asarray(kv, dtype=np.float32)
    in_maps = []
    for b in range(B):
        m = dict(shared)
        m["qT"] = f16(qn[b].T)
        m["kvT"] = f16(kvn[b].T)
        in_maps.append(m)
    return in_maps


def kernel(q, kv, Wq, Wkv, Wproj, bproj, tau, _trace=False, _knobs=None):
    B, LQ, C = q.shape
    LKV = kv.shape[1]
    H = 16 if C == 1024 else max(1, C // 64)
    assert B == NCORES, f"expected B == {NCORES}, got {B}"

    nc = _get_nc(C, H, LQ, LKV, knobs=_knobs)
    in_maps = _host_inputs(q, kv, Wq, Wkv, Wproj, bproj, tau, H)
    res = run_bass_kernel_spmd(
        nc, in_maps, core_ids=list(range(NCORES)), trace=_trace
    )
    bp = np.asarray(bproj, dtype=np.float64).reshape(1, C)
    out = np.stack(
        [res.results[b]["y"].astype(np.float64) + bp for b in range(B)], axis=0
    )
    out = out.astype(np.asarray(q).dtype)
    if _trace:
        kernel._last_result = res
    return out
